# revision 8
# baseline (speedup 1.0000x reference)
"""Trainium2 Bass kernel for nn_DilationLayerExtSE (morphological dilation,
external structuring element, per-sample/per-channel weights).

    out[b,c,i,j] = max_{di,dj} (xpad[b,c,i+di,j+dj] + weight[b,c,di,dj]) + bias[b,c]

Shapes (hardcoded): x (8,128,128,128) f32, weight (8,128,5,5) f32,
bias (8,128) f32, padding=2, stride=1 -> out (8,128,128,128) f32.

Sharding: data-parallel over B across the 8 NeuronCores (1 sample/core).
Per core: C=128 maps onto the 128 SBUF partitions; each channel's padded
132x132 plane is a flat 17424-element stream in that partition.

This version computes in bf16 (harness gate is rel_err < 2e-2; bf16 keeps it
~1e-3).  The DVE runs 2-byte dtypes in its high-rate perf mode, roughly
halving the per-pass cost vs fp32; it is the only engine this toolchain
allows to run tensor-tensor ALU ops (the Pool/GPSIMD engine rejects
TensorTensor/TensorScalarPtr in the V3 ISA check, and DMA accum-max is
rejected too), so all 24 fused max-plus passes live on DVE.

I/O is bulk: x is loaded as one contiguous 32KB/partition stream per band
(1 descriptor per partition instead of one per 256B row), ACT re-lays it
into the 132-pitch padded buffer, and the last max-plus pass writes its
result packed (128-pitch) so the store is bulk too.  bias is folded into
the 25 SE weights on the host (max_k(p+w_k)+b == max_k(p+(w_k+b))).
"""

import os
import time

import numpy as np
import ml_dtypes

B, C, H, W = 8, 128, 128, 128
KH = KW = 5
PAD = 2
HP, WP = H + 2 * PAD, W + 2 * PAD  # 132, 132
NK = KH * KW
XLEN = HP * WP + 8  # flat padded plane + tail so the last band's k=24 slice is in-bounds

# Row-band sizes: small first band lets DVE start early; small last band
# leaves only a tiny store after the final pass.
LANES = os.environ.get("KERNEL_LANES", "8,56,56,8")
PRELAYOUT = int(os.environ.get("KERNEL_PRELAYOUT", "1"))
PACKOUT = int(os.environ.get("KERNEL_PACKOUT", "1"))
NARROW = int(os.environ.get("KERNEL_NARROW", "0"))
NITER = int(os.environ.get("KERNEL_NITER", "0"))

_CACHE: dict = {}

LAST_RUN_SECONDS: float | None = None
LAST_EXEC_TIME_NS: int | None = None


def _parse_bands():
    bands = []
    r0 = 0
    for part in LANES.split(","):
        rows = int(part.lstrip("v"))
        bands.append((r0, rows))
        r0 += rows
    assert r0 == H, f"bands must cover {H} rows, got {r0}"
    return bands


def _build_program():
    from contextlib import ExitStack

    import concourse.bacc as bacc
    import concourse.tile as tile
    from concourse import mybir

    bands = _parse_bands()

    nc = bacc.Bacc("TRN2", target_bir_lowering=False, debug=False)
    bf = mybir.dt.bfloat16
    f32 = mybir.dt.float32
    x = nc.dram_tensor("x", [C, H * W], bf, kind="ExternalInput")
    w = nc.dram_tensor("w", [C, NK], f32, kind="ExternalInput")
    out = nc.dram_tensor("out", [C, H * W], bf, kind="ExternalOutput")

    add = mybir.AluOpType.add
    mx = mybir.AluOpType.max
    ident = mybir.ActivationFunctionType.Identity

    with tile.TileContext(nc) as tc, ExitStack() as ctx:
        const = ctx.enter_context(tc.tile_pool(name="const", bufs=1))
        acc_p = ctx.enter_context(tc.tile_pool(name="acc", bufs=2))
        outp_p = ctx.enter_context(tc.tile_pool(name="outp", bufs=2))

        xpad = const.tile([C, XLEN], bf)
        wb = const.tile([C, NK], f32)
        xflat = const.tile([C, H * W], bf, name="xflat") if PRELAYOUT else None

        xp3 = xpad[:, 0 : HP * WP].rearrange("c (h w) -> c h w", w=WP)
        # zero the pad borders + tail (interior is overwritten per band)
        nc.gpsimd.memset(xpad[:, 0 : PAD * WP], 0.0)
        nc.gpsimd.memset(xpad[:, (HP - PAD) * WP : XLEN], 0.0)
        nc.gpsimd.memset(xp3[:, PAD : HP - PAD, 0:PAD], 0.0)
        nc.gpsimd.memset(xp3[:, PAD : HP - PAD, WP - PAD : WP], 0.0)

        nc.sync.dma_start(out=wb[:], in_=w[:, :])

        def load_band(r0, rows):
            if PRELAYOUT:
                nc.sync.dma_start(
                    out=xflat[:, r0 * W : (r0 + rows) * W],
                    in_=x[:, r0 * W : (r0 + rows) * W],
                )
            else:
                nc.sync.dma_start(
                    out=xp3[:, PAD + r0 : PAD + r0 + rows, PAD : PAD + W],
                    in_=x[:, r0 * W : (r0 + rows) * W].rearrange(
                        "c (h w) -> c h w", w=W
                    ),
                )

        RCHUNK = 16  # relayout granularity (rows)

        def relayout_chunk(r0, rows):
            src = xflat[:, r0 * W : (r0 + rows) * W].rearrange(
                "c (h w) -> c h w", w=W
            )
            nc.scalar.copy(xp3[:, PAD + r0 : PAD + r0 + rows, PAD : PAD + W], src)

        def body(_iv=None):
            for r0, rows in bands:
                load_band(r0, rows)
            # Just-in-time chunked relayout on ACT: a band's chain reads xpad
            # interior rows < band_end + 2 (di up to 4), so every chunk
            # covering those rows must be EMITTED before the chain's reads —
            # program order defines the dataflow Tile enforces.
            emitted = [0]  # rows relayed so far

            def ensure_relayout(upto):
                while PRELAYOUT and emitted[0] < min(upto, H):
                    rows = min(RCHUNK, H - emitted[0])
                    relayout_chunk(emitted[0], rows)
                    emitted[0] += rows

            for r0, rows in bands:
                ensure_relayout(r0 + rows + PAD)
                L = rows * WP

                def win(k, _r0=r0, _L=L):
                    di, dj = divmod(k, KW)
                    base = (_r0 + di) * WP + dj
                    return xpad[:, base : base + _L]

                def win3(k, _r0=r0, _rows=rows):
                    di, dj = divmod(k, KW)
                    base = (_r0 + di) * WP + dj
                    return xpad[:, base : base + _rows * WP].rearrange(
                        "c (h w) -> c h w", w=WP
                    )[:, :, 0:W]

                if NARROW:
                    # packed accumulator (pitch W): every pass is [rows, W]
                    # 2D-strided on the window side, packed on the acc side;
                    # the store DMAs straight from acc.
                    acc = acc_p.tile([C, rows, W], bf, tag="acc")
                    nc.scalar.activation(
                        acc[:, :, :], win3(0), ident, bias=wb[:, 0:1], scale=1.0
                    )
                    for k in range(1, NK):
                        nc.vector.scalar_tensor_tensor(
                            out=acc[:, :, :], in0=win3(k), scalar=wb[:, k : k + 1],
                            in1=acc[:, :, :], op0=add, op1=mx,
                        )
                    nc.sync.dma_start(
                        out=out[:, r0 * W : (r0 + rows) * W],
                        in_=acc.rearrange("c h w -> c (h w)")[:, :],
                    )
                    continue

                acc = acc_p.tile([C, L], bf, tag="acc")
                acc3 = acc.rearrange("c (h w) -> c h w", w=WP)
                # k = 0 seeds the accumulator on ACT: acc = x_win + wb[0]
                nc.scalar.activation(
                    acc[:], win(0), ident, bias=wb[:, 0:1], scale=1.0
                )
                last = NK - 1 if PACKOUT else NK
                for k in range(1, last):
                    nc.vector.scalar_tensor_tensor(
                        out=acc[:], in0=win(k), scalar=wb[:, k : k + 1],
                        in1=acc[:], op0=add, op1=mx,
                    )
                if PACKOUT:
                    # final pass writes packed so the store is bulk
                    outp = outp_p.tile([C, rows, W], bf, tag="outp")
                    nc.vector.scalar_tensor_tensor(
                        out=outp[:, :, :], in0=win3(NK - 1), scalar=wb[:, NK - 1 : NK],
                        in1=acc3[:, :, 0:W], op0=add, op1=mx,
                    )
                    nc.sync.dma_start(
                        out=out[:, r0 * W : (r0 + rows) * W],
                        in_=outp.rearrange("c h w -> c (h w)")[:, :],
                    )
                else:
                    nc.sync.dma_start(
                        out=out[:, r0 * W : (r0 + rows) * W].rearrange(
                            "c (h w) -> c h w", w=W
                        ),
                        in_=acc3[:, :, 0:W],
                    )

        if NITER > 0:
            with tc.For_i(0, NITER, 1):
                body()
        else:
            body()

    nc.compile()
    return nc


def _get_nc():
    key = (LANES, PRELAYOUT, NARROW, NITER)
    if key not in _CACHE:
        _CACHE[key] = _build_program()
    return _CACHE[key]


def kernel(x, weight, bias, padding, stride):
    global LAST_RUN_SECONDS, LAST_EXEC_TIME_NS
    from concourse.bass_utils import run_bass_kernel_spmd

    x = np.asarray(x, dtype=np.float32)
    weight = np.asarray(weight, dtype=np.float32)
    bias = np.asarray(bias, dtype=np.float32)
    assert int(padding) == PAD and int(stride) == 1
    assert x.shape == (B, C, H, W) and weight.shape == (B, C, KH, KW)

    nc = _get_nc()
    xb = x.reshape(B, C, H * W).astype(ml_dtypes.bfloat16)
    wb = (weight.reshape(B, C, NK) + bias[:, :, None]).astype(np.float32)
    in_maps = [
        {
            "x": np.ascontiguousarray(xb[i]),
            "w": np.ascontiguousarray(wb[i]),
        }
        for i in range(B)
    ]
    t0 = time.perf_counter()
    res = run_bass_kernel_spmd(nc, in_maps, core_ids=list(range(B)))
    LAST_RUN_SECONDS = time.perf_counter() - t0
    LAST_EXEC_TIME_NS = res.exec_time_ns
    return np.stack(
        [
            np.asarray(res.results[i]["out"])
            .astype(np.float32)
            .reshape(C, H, W)
            for i in range(B)
        ],
        axis=0,
    )


# revision 13
# speedup vs baseline: 1.1498x; 1.1498x over previous
"""Trainium2 Bass kernel for nn_DilationLayerExtSE (morphological dilation,
external structuring element, per-sample/per-channel weights).

    out[b,c,i,j] = max_{di,dj} (xpad[b,c,i+di,j+dj] + weight[b,c,di,dj]) + bias[b,c]

Shapes (hardcoded): x (8,128,128,128) f32, weight (8,128,5,5) f32,
bias (8,128) f32, padding=2, stride=1 -> out (8,128,128,128) f32.

Sharding: data-parallel over B across the 8 NeuronCores (1 sample/core).
Per core: C=128 maps onto the 128 SBUF partitions; each channel's padded
132x132 plane lives in that partition; bias is folded into the 25 SE
weights on the host (max_k(p+w_k)+b == max_k(p+(w_k+b))).

Engine facts measured on this silicon (per-elem, bf16 SBUF):
  - scalar_tensor_tensor (fused add+max): 1.0 ns — only a 1x uop exists.
  - tensor_tensor (max):                  0.54 ns — 2x_1p mode.
  - tensor_scalar (add, +per-part scalar):0.26 ns on DVE — 4x mode.
  - ACT activation (add via bias):        0.84 ns, dtype-independent.
  - Pool/GPSIMD tensor_scalar:            14 ns — useless; and the V3 ISA
    check rejects every tensor-tensor ALU op on Pool, plus DMA accum-max.

So the optimal split: DVE runs the 24 max passes as pure tensor_tensor at
2x on packed bf16 planes, while the 25 `x_win + w_k` tmp planes are
produced 17 on ACT (activation-identity with per-partition bias) and 8 on
DVE itself with 4x tensor_scalar (the k=0 feed seeds the accumulator
directly).  Accumulator and tmp planes are packed at pitch 128 (windows are
read [rows,128]-strided out of the 132-pitch padded plane), so the max
passes stream fully packed and the store DMA is bulk (1 descriptor per
partition per band).
"""

import os
import time

import numpy as np
import ml_dtypes

B, C, H, W = 8, 128, 128, 128
KH = KW = 5
PAD = 2
HP, WP = H + 2 * PAD, W + 2 * PAD  # 132, 132
NK = KH * KW
XLEN = HP * WP + 8

# Row-band sizes: small first band primes the pipeline.
LANES = os.environ.get("KERNEL_LANES", "8,40,40,40")
# ks whose tmp feed runs on DVE's 4x tensor_scalar (even dj, spread over di);
# k=0 doubles as the accumulator seed.  The other 17 feeds run on ACT.
DVE_FEED = frozenset(
    int(s) for s in os.environ.get("KERNEL_DVEFEED", "0,2,4,10,12,14,20,22").split(",")
)
TMP_BUFS = int(os.environ.get("KERNEL_TMPBUFS", "5"))
NITER = int(os.environ.get("KERNEL_NITER", "0"))

_CACHE: dict = {}

LAST_RUN_SECONDS: float | None = None
LAST_EXEC_TIME_NS: int | None = None


def _parse_bands():
    bands = []
    r0 = 0
    for part in LANES.split(","):
        rows = int(part.lstrip("v"))
        bands.append((r0, rows))
        r0 += rows
    assert r0 == H, f"bands must cover {H} rows, got {r0}"
    return bands


def _build_program():
    from contextlib import ExitStack

    import concourse.bacc as bacc
    import concourse.tile as tile
    from concourse import mybir

    bands = _parse_bands()

    nc = bacc.Bacc("TRN2", target_bir_lowering=False, debug=False)
    bf = mybir.dt.bfloat16
    f32 = mybir.dt.float32
    x = nc.dram_tensor("x", [C, H * W], bf, kind="ExternalInput")
    w = nc.dram_tensor("w", [C, NK], f32, kind="ExternalInput")
    out = nc.dram_tensor("out", [C, H * W], bf, kind="ExternalOutput")

    add = mybir.AluOpType.add
    mx = mybir.AluOpType.max
    ident = mybir.ActivationFunctionType.Identity

    with tile.TileContext(nc) as tc, ExitStack() as ctx:
        const = ctx.enter_context(tc.tile_pool(name="const", bufs=1))
        acc_p = ctx.enter_context(tc.tile_pool(name="acc", bufs=2))
        tmp_p = ctx.enter_context(tc.tile_pool(name="tmp", bufs=TMP_BUFS))

        xpad = const.tile([C, XLEN], bf)
        wb = const.tile([C, NK], f32)

        xp3 = xpad[:, 0 : HP * WP].rearrange("c (h w) -> c h w", w=WP)
        # zero the pad borders + tail (interior is overwritten per band)
        nc.gpsimd.memset(xpad[:, 0 : PAD * WP], 0.0)
        nc.gpsimd.memset(xpad[:, (HP - PAD) * WP : XLEN], 0.0)
        nc.gpsimd.memset(xp3[:, PAD : HP - PAD, 0:PAD], 0.0)
        nc.gpsimd.memset(xp3[:, PAD : HP - PAD, WP - PAD : WP], 0.0)

        nc.sync.dma_start(out=wb[:], in_=w[:, :])

        def body(_iv=None):
            for r0, rows in bands:
                nc.sync.dma_start(
                    out=xp3[:, PAD + r0 : PAD + r0 + rows, PAD : PAD + W],
                    in_=x[:, r0 * W : (r0 + rows) * W].rearrange(
                        "c (h w) -> c h w", w=W
                    ),
                )
            for r0, rows in bands:

                def win3(k, _r0=r0, _rows=rows):
                    di, dj = divmod(k, KW)
                    base = (_r0 + di) * WP + dj
                    return xpad[:, base : base + _rows * WP].rearrange(
                        "c (h w) -> c h w", w=WP
                    )[:, :, 0:W]

                acc = acc_p.tile([C, rows, W], bf, tag="acc")
                accf = acc.rearrange("c h w -> c (h w)")

                def feed(k, dst3):
                    if k in DVE_FEED:
                        nc.vector.tensor_scalar(
                            out=dst3[:, :, :], in0=win3(k),
                            scalar1=wb[:, k : k + 1], scalar2=None, op0=add,
                        )
                    else:
                        nc.scalar.activation(
                            dst3[:, :, :], win3(k), ident,
                            bias=wb[:, k : k + 1], scale=1.0,
                        )

                feed(0, acc)  # seed
                for k in range(1, NK):
                    tmp = tmp_p.tile([C, rows, W], bf, tag="tmp")
                    feed(k, tmp)
                    tmpf = tmp.rearrange("c h w -> c (h w)")
                    nc.vector.tensor_tensor(
                        out=accf[:, :], in0=tmpf[:, :], in1=accf[:, :], op=mx
                    )
                nc.sync.dma_start(
                    out=out[:, r0 * W : (r0 + rows) * W], in_=accf[:, :]
                )

        if NITER > 0:
            with tc.For_i(0, NITER, 1):
                body()
        else:
            body()

    nc.compile()
    return nc


def _get_nc():
    key = (LANES, NITER)
    if key not in _CACHE:
        _CACHE[key] = _build_program()
    return _CACHE[key]


def kernel(x, weight, bias, padding, stride):
    global LAST_RUN_SECONDS, LAST_EXEC_TIME_NS
    from concourse.bass_utils import run_bass_kernel_spmd

    x = np.asarray(x, dtype=np.float32)
    weight = np.asarray(weight, dtype=np.float32)
    bias = np.asarray(bias, dtype=np.float32)
    assert int(padding) == PAD and int(stride) == 1
    assert x.shape == (B, C, H, W) and weight.shape == (B, C, KH, KW)

    nc = _get_nc()
    xb = x.reshape(B, C, H * W).astype(ml_dtypes.bfloat16)
    wb = (weight.reshape(B, C, NK) + bias[:, :, None]).astype(np.float32)
    in_maps = [
        {
            "x": np.ascontiguousarray(xb[i]),
            "w": np.ascontiguousarray(wb[i]),
        }
        for i in range(B)
    ]
    t0 = time.perf_counter()
    res = run_bass_kernel_spmd(nc, in_maps, core_ids=list(range(B)))
    LAST_RUN_SECONDS = time.perf_counter() - t0
    LAST_EXEC_TIME_NS = res.exec_time_ns
    return np.stack(
        [
            np.asarray(res.results[i]["out"])
            .astype(np.float32)
            .reshape(C, H, W)
            for i in range(B)
        ],
        axis=0,
    )


# revision 14
# speedup vs baseline: 1.2320x; 1.0715x over previous
"""Trainium2 Bass kernel for nn_DilationLayerExtSE (morphological dilation,
external structuring element, per-sample/per-channel weights).

    out[b,c,i,j] = max_{di,dj} (xpad[b,c,i+di,j+dj] + weight[b,c,di,dj]) + bias[b,c]

Shapes (hardcoded): x (8,128,128,128) f32, weight (8,128,5,5) f32,
bias (8,128) f32, padding=2, stride=1 -> out (8,128,128,128) f32.

Sharding: data-parallel over B across the 8 NeuronCores (1 sample/core).
Per core: C=128 maps onto the 128 SBUF partitions; each channel's padded
132x132 plane lives in that partition; bias is folded into the 25 SE
weights on the host (max_k(p+w_k)+b == max_k(p+(w_k+b))).

Engine facts measured on this silicon (per-elem, bf16 SBUF):
  - scalar_tensor_tensor (fused add+max): 1.0 ns — only a 1x uop exists.
  - tensor_tensor (max):                  0.54 ns — 2x_1p mode.
  - tensor_scalar (add, +per-part scalar):0.26 ns on DVE — 4x mode.
  - ACT activation (add via bias):        0.84 ns, dtype-independent.
  - Pool/GPSIMD tensor_scalar:            14 ns — useless; and the V3 ISA
    check rejects every tensor-tensor ALU op on Pool, plus DMA accum-max.

So the optimal split: DVE runs the 24 max passes as pure tensor_tensor at
2x on packed bf16 planes, while the 25 `x_win + w_k` tmp planes are
produced 17 on ACT (activation-identity with per-partition bias) and 8 on
DVE itself with 4x tensor_scalar (the k=0 feed seeds the accumulator
directly).  Accumulator and tmp planes are packed at pitch 128 (windows are
read [rows,128]-strided out of the 132-pitch padded plane), so the max
passes stream fully packed and the store DMA is bulk (1 descriptor per
partition per band).
"""

import os
import time

import numpy as np
import ml_dtypes

B, C, H, W = 8, 128, 128, 128
KH = KW = 5
PAD = 2
HP, WP = H + 2 * PAD, W + 2 * PAD  # 132, 132
NK = KH * KW
XLEN = HP * WP + 8

# Row-band sizes: small first band primes the pipeline.
LANES = os.environ.get("KERNEL_LANES", "8,40,40,40")
# ks whose tmp feed runs on DVE's 4x tensor_scalar (even dj, spread over di);
# k=0 doubles as the accumulator seed.  The other 17 feeds run on ACT.
DVE_FEED = frozenset(
    int(s) for s in os.environ.get("KERNEL_DVEFEED", "0,2,4,10,12,14,20,22").split(",")
)
TMP_BUFS = int(os.environ.get("KERNEL_TMPBUFS", "5"))
NOCOMPUTE = int(os.environ.get("KERNEL_NOCOMPUTE", "0"))
NITER = int(os.environ.get("KERNEL_NITER", "0"))

_CACHE: dict = {}

LAST_RUN_SECONDS: float | None = None
LAST_EXEC_TIME_NS: int | None = None


def _parse_bands():
    bands = []
    r0 = 0
    for part in LANES.split(","):
        rows = int(part.lstrip("v"))
        bands.append((r0, rows))
        r0 += rows
    assert r0 == H, f"bands must cover {H} rows, got {r0}"
    return bands


def _build_program():
    from contextlib import ExitStack

    import concourse.bacc as bacc
    import concourse.tile as tile
    from concourse import mybir

    bands = _parse_bands()

    nc = bacc.Bacc("TRN2", target_bir_lowering=False, debug=False)
    bf = mybir.dt.bfloat16
    f32 = mybir.dt.float32
    x = nc.dram_tensor("x", [C, H * W], bf, kind="ExternalInput")
    w = nc.dram_tensor("w", [C, NK], f32, kind="ExternalInput")
    out = nc.dram_tensor("out", [C, H * W], bf, kind="ExternalOutput")

    add = mybir.AluOpType.add
    mx = mybir.AluOpType.max
    ident = mybir.ActivationFunctionType.Identity

    with tile.TileContext(nc) as tc, ExitStack() as ctx:
        const = ctx.enter_context(tc.tile_pool(name="const", bufs=1))
        acc_p = ctx.enter_context(tc.tile_pool(name="acc", bufs=2))
        tmp_p = ctx.enter_context(tc.tile_pool(name="tmp", bufs=TMP_BUFS))

        xpad = const.tile([C, XLEN], bf)
        wb = const.tile([C, NK], f32)

        xp3 = xpad[:, 0 : HP * WP].rearrange("c (h w) -> c h w", w=WP)
        # zero the pad borders + tail (interior is overwritten per band)
        nc.gpsimd.memset(xpad[:, 0 : PAD * WP], 0.0)
        nc.gpsimd.memset(xpad[:, (HP - PAD) * WP : XLEN], 0.0)
        nc.gpsimd.memset(xp3[:, PAD : HP - PAD, 0:PAD], 0.0)
        nc.gpsimd.memset(xp3[:, PAD : HP - PAD, WP - PAD : WP], 0.0)

        nc.sync.dma_start(out=wb[:], in_=w[:, :])

        def body(_iv=None):
            for r0, rows in bands:
                nc.sync.dma_start(
                    out=xp3[:, PAD + r0 : PAD + r0 + rows, PAD : PAD + W],
                    in_=x[:, r0 * W : (r0 + rows) * W].rearrange(
                        "c (h w) -> c h w", w=W
                    ),
                )
            for r0, rows in bands:

                def win3(k, _r0=r0, _rows=rows):
                    di, dj = divmod(k, KW)
                    base = (_r0 + di) * WP + dj
                    return xpad[:, base : base + _rows * WP].rearrange(
                        "c (h w) -> c h w", w=WP
                    )[:, :, 0:W]

                acc = acc_p.tile([C, rows, W], bf, tag="acc")
                accf = acc.rearrange("c h w -> c (h w)")

                def feed(k, dst3):
                    if k in DVE_FEED:
                        nc.vector.tensor_scalar(
                            out=dst3[:, :, :], in0=win3(k),
                            scalar1=wb[:, k : k + 1], scalar2=None, op0=add,
                        )
                    else:
                        nc.scalar.activation(
                            dst3[:, :, :], win3(k), ident,
                            bias=wb[:, k : k + 1], scale=1.0,
                        )

                feed(0, acc)  # seed
                for k in range(1, 1 if NOCOMPUTE else NK):
                    tmp = tmp_p.tile([C, rows, W], bf, tag="tmp")
                    feed(k, tmp)
                    tmpf = tmp.rearrange("c h w -> c (h w)")
                    nc.vector.tensor_tensor(
                        out=accf[:, :], in0=tmpf[:, :], in1=accf[:, :], op=mx
                    )
                nc.sync.dma_start(
                    out=out[:, r0 * W : (r0 + rows) * W], in_=accf[:, :]
                )

        if NITER > 0:
            with tc.For_i(0, NITER, 1):
                body()
        else:
            body()

    nc.compile()
    return nc


def _get_nc():
    key = (LANES, NITER)
    if key not in _CACHE:
        _CACHE[key] = _build_program()
    return _CACHE[key]


def kernel(x, weight, bias, padding, stride):
    global LAST_RUN_SECONDS, LAST_EXEC_TIME_NS
    from concourse.bass_utils import run_bass_kernel_spmd

    x = np.asarray(x, dtype=np.float32)
    weight = np.asarray(weight, dtype=np.float32)
    bias = np.asarray(bias, dtype=np.float32)
    assert int(padding) == PAD and int(stride) == 1
    assert x.shape == (B, C, H, W) and weight.shape == (B, C, KH, KW)

    nc = _get_nc()
    xb = x.reshape(B, C, H * W).astype(ml_dtypes.bfloat16)
    wb = (weight.reshape(B, C, NK) + bias[:, :, None]).astype(np.float32)
    in_maps = [
        {
            "x": np.ascontiguousarray(xb[i]),
            "w": np.ascontiguousarray(wb[i]),
        }
        for i in range(B)
    ]
    t0 = time.perf_counter()
    res = run_bass_kernel_spmd(nc, in_maps, core_ids=list(range(B)))
    LAST_RUN_SECONDS = time.perf_counter() - t0
    LAST_EXEC_TIME_NS = res.exec_time_ns
    return np.stack(
        [
            np.asarray(res.results[i]["out"])
            .astype(np.float32)
            .reshape(C, H, W)
            for i in range(B)
        ],
        axis=0,
    )
